# revision 21
# baseline (speedup 1.0000x reference)
"""Multi-head self-attention on 8 Trainium2 NeuronCores.

Sharding: core c = b*4 + g handles batch b (of 2) and head-group g (4 heads
of 16). Per core: full qkv projection for its 4 heads, attention, and a
partial output projection (row-slice of Wout). Host sums the 4 partials per
batch and adds bout.

The PE sustains ~0.833 ns per moving row regardless of dtype, so the kernel
is PE-row-bound: q/k projection streams x as moving rows, v projection swaps
operands (x token-tile stationary, Wv moving) so V lands directly in
[token, head-dim] layout without PE transposes. Attention runs as one flat
software-pipelined stream over (head, query-pair, key-tile) jobs: scores for
job i, one [128,1024] ACT exp, then the PV accumulation for job i-3, keeping
the PE gapless while exp latency hides. Softmax denominators ride as a
ones-column through PV; reciprocals come from one fast-approx DVE op per
pair and normalization happens in SBUF on otherwise-idle engines.
"""

import os
from contextlib import ExitStack

import ml_dtypes
import numpy as np

import concourse.bass as bass
import concourse.bacc as bacc
import concourse.tile as tile
from concourse import mybir
from concourse._compat import with_exitstack
from concourse.bass_utils import run_bass_kernel_spmd

B, S, E, H = 2, 2048, 1024, 16
HD = 64
SCALE = HD ** -0.5
NCORES = 8
GROUPS = 4                 # head-groups per batch == cores per batch
HPG = H // GROUPS          # 4 heads per core
DG = HPG * HD              # 256 qkv cols per core per projection
KC = E // 128              # 8 contraction chunks
NT = S // 512              # 4 query chunks of 512
SKT = S // 128             # 16 key tiles of 128
VBLK = 65                  # v block cols: 64 v dims + ones column
DEPTH = 3                  # attention software-pipeline depth

FP = mybir.dt.float32
FR = mybir.dt.float32r
BF = mybir.dt.bfloat16


@with_exitstack
def _mha_body(ctx: ExitStack, tc: tile.TileContext, xt, w, bqkv, wo, y):
    nc = tc.nc
    main = ctx.enter_context(tc.tile_pool(name="main", bufs=1))

    qT = [main.tile([128, S], FR, name=f"qT{p}") for p in range(2)]
    kT = [main.tile([128, S], FR, name=f"kT{p}") for p in range(2)]
    v_store = main.tile([128, SKT * HPG * VBLK], FR)   # [128, 4160]
    attn = [main.tile([128, S], FR, name=f"attn{p}") for p in range(2)]
    wo_sb = [main.tile([128, E], FR, name=f"wo{p}") for p in range(2)]
    b_sb = main.tile([128, 4], FP)
    vb = main.tile([128, DG], FP)
    den_all = main.tile([1, 16 * 512], FP)
    rden_all = main.tile([1, 16 * 512], FP)

    for m in range(4):
        nc.gpsimd.dma_start(out=b_sb[:, m : m + 1], in_=bqkv[m * 128 : (m + 1) * 128, :])
    vb_row = main.tile([1, DG], FP)
    nc.gpsimd.dma_start(out=vb_row, in_=bqkv[512:768, :])
    nc.gpsimd.partition_broadcast(vb, vb_row)

    vs_view = v_store.rearrange("p (j c) -> p j c", c=VBLK)

    # ---- phase A: qkv projection ----
    with tc.tile_pool(name="xw", bufs=1) as xw:
        xts = [xw.tile([128, S], BF, name=f"xts{k}") for k in range(KC)]
        wts = [xw.tile([128, 768], BF, name=f"wts{k}") for k in range(KC)]
        for k in range(KC):
            nc.default_dma_engine.dma_start(out=xts[k], in_=xt[k * 128 : (k + 1) * 128, :])
            nc.sync.dma_start(out=wts[k], in_=w[k * 128 : (k + 1) * 128, :])

        # ones columns for the denominator trick
        ones_src = xw.tile([128, SKT * HPG], FP)
        nc.vector.memset(ones_src, 1.0)
        nc.vector.tensor_copy(
            vs_view[:, :, 64:65], ones_src.rearrange("p (j c) -> p j c", c=1)
        )

        dsts = [qT[0], qT[1], kT[0], kT[1]]
        with tc.tile_pool(name="qk_ps", bufs=2, space="PSUM") as qk_ps:
            for m in range(4):
                pss = [qk_ps.tile([128, 512], FP, name=f"qps{n}") for n in range(NT)]
                for k in range(KC):
                    for n in range(NT):
                        nc.tensor.matmul(
                            pss[n],
                            wts[k][:, m * 128 : (m + 1) * 128],
                            xts[k][:, n * 512 : (n + 1) * 512],
                            start=(k == 0),
                            stop=(k == KC - 1),
                        )
                for n in range(NT):
                    nc.vector.tensor_scalar_add(
                        dsts[m][:, n * 512 : (n + 1) * 512], pss[n], b_sb[:, m : m + 1]
                    )

        # v projection: token tile stationary so out is [token, vcol]
        with tc.tile_pool(name="v_ps", bufs=4, space="PSUM") as v_ps:
            for tt in range(SKT):
                vp = v_ps.tile([128, DG], FP, name="vps")
                for k in range(KC):
                    nc.tensor.matmul(
                        vp,
                        xts[k][:, tt * 128 : (tt + 1) * 128],
                        wts[k][:, 512:768],
                        start=(k == 0),
                        stop=(k == KC - 1),
                    )
                nc.vector.tensor_add(
                    vs_view[:, tt * HPG : (tt + 1) * HPG, 0:64],
                    vp.rearrange("p (j c) -> p j c", c=64),
                    vb.rearrange("p (j c) -> p j c", c=64),
                )

    # preload Wout during attention
    for p in range(2):
        nc.default_dma_engine.dma_start(out=wo_sb[p], in_=wo[p * 128 : (p + 1) * 128, :])

    # ---- phase B: attention, one flat pipelined stream ----
    with tc.tile_pool(name="sc_ps", bufs=2, space="PSUM") as sc_ps, \
         tc.tile_pool(name="pv_ps", bufs=2, space="PSUM") as pv_ps, \
         tc.tile_pool(name="probs", bufs=DEPTH + 3) as probs_pool, \
         tc.tile_pool(name="bcast", bufs=3) as bcast_pool:
        jobs = [(h, np_, t) for h in range(HPG) for np_ in range(2) for t in range(SKT)]
        pr_slots = [None] * len(jobs)
        atts = None
        for i in range(len(jobs) + DEPTH):
            if i < len(jobs):
                h, np_, t = jobs[i]
                pi, off = h >> 1, (h & 1) * 64
                s2 = sc_ps.tile([128, 1024], FP, name="s2")
                for j in range(2):
                    nq = np_ * 2 + j
                    nc.tensor.matmul(
                        s2[:, j * 512 : (j + 1) * 512],
                        kT[pi][off : off + 64, t * 128 : (t + 1) * 128],
                        qT[pi][off : off + 64, nq * 512 : (nq + 1) * 512],
                        start=True,
                        stop=True,
                    )
                pr2 = probs_pool.tile([128, 1024], FR, name="pr2")
                nc.scalar.activation(
                    pr2, s2, mybir.ActivationFunctionType.Exp, scale=SCALE
                )
                pr_slots[i] = pr2
            io = i - DEPTH
            if io >= 0:
                h0, np0, t0 = jobs[io]
                pi0, off0 = h0 >> 1, (h0 & 1) * 64
                if t0 == 0:
                    atts = [pv_ps.tile([VBLK, 512], FP, name=f"att{j}") for j in range(2)]
                pr0 = pr_slots[io]
                pr_slots[io] = None
                blk = (t0 * HPG + h0) * VBLK
                for j in range(2):
                    nc.tensor.matmul(
                        atts[j],
                        v_store[:, blk : blk + VBLK],
                        pr0[:, j * 512 : (j + 1) * 512],
                        start=(t0 == 0),
                        stop=(t0 == SKT - 1),
                    )
                if t0 == SKT - 1:
                    # drain pair: unnormalized attn rows + denominators
                    r0 = h0 * NT + np0 * 2
                    for j in range(2):
                        nq = np0 * 2 + j
                        nc.vector.tensor_copy(
                            attn[pi0][off0 : off0 + 64, nq * 512 : (nq + 1) * 512],
                            atts[j][0:64, :],
                        )
                        nc.vector.tensor_copy(
                            den_all[:, (r0 + j) * 512 : (r0 + j + 1) * 512],
                            atts[j][64:65, :],
                        )
                    nc.vector.reciprocal_approx_fast(
                        rden_all[:, r0 * 512 : (r0 + 2) * 512],
                        den_all[:, r0 * 512 : (r0 + 2) * 512],
                    )
                    for j in range(2):
                        nq = np0 * 2 + j
                        rden128 = bcast_pool.tile([128, 512], FP, name="rb")
                        nc.gpsimd.partition_broadcast(
                            rden128, rden_all[:, (r0 + j) * 512 : (r0 + j + 1) * 512]
                        )
                        sl = attn[pi0][off0 : off0 + 64, nq * 512 : (nq + 1) * 512]
                        nc.vector.tensor_mul(sl, sl, rden128[off0 : off0 + 64, :])

    # ---- phase C: output projection (partial; host sums over groups) ----
    with tc.tile_pool(name="y_ps", bufs=4, space="PSUM") as y_ps, \
         tc.tile_pool(name="y_sb", bufs=4) as y_sb:
        for mt in range(SKT):
            for n2 in range(2):
                ps = y_ps.tile([128, 512], FP)
                for p in range(2):
                    nc.tensor.matmul(
                        ps,
                        attn[p][:, mt * 128 : (mt + 1) * 128],
                        wo_sb[p][:, n2 * 512 : (n2 + 1) * 512],
                        start=(p == 0),
                        stop=(p == 1),
                    )
                yt = y_sb.tile([128, 512], BF)
                if n2 == 0:
                    nc.vector.tensor_copy(yt, ps)
                else:
                    nc.scalar.copy(yt, ps)
                nc.default_dma_engine.dma_start(
                    out=y[mt * 128 : (mt + 1) * 128, n2 * 512 : (n2 + 1) * 512], in_=yt
                )


_PROGRAM = None


def _get_program():
    global _PROGRAM
    if _PROGRAM is None:
        nc = bacc.Bacc(
            "TRN2",
            target_bir_lowering=False,
            debug=False,
            enable_asserts=False,
            num_devices=NCORES,
        )
        xt = nc.dram_tensor("xt", [E, S], BF, kind="ExternalInput").ap()
        w = nc.dram_tensor("wqkv", [E, 768], BF, kind="ExternalInput").ap()
        bq = nc.dram_tensor("bqkv", [768, 1], FP, kind="ExternalInput").ap()
        wo = nc.dram_tensor("wout", [DG, E], FR, kind="ExternalInput").ap()
        y = nc.dram_tensor("y", [S, E], BF, kind="ExternalOutput").ap()
        with tile.TileContext(nc) as tc:
            _mha_body(tc, xt, w, bq, wo, y)
        nc.compile()
        _PROGRAM = nc
    return _PROGRAM


LAST_RESULTS = None


def kernel(x, Wqkv, bqkv, Wout, bout):
    global LAST_RESULTS
    x = np.asarray(x, np.float32)
    Wqkv = np.asarray(Wqkv, np.float32)
    bqkv = np.asarray(bqkv, np.float32)
    Wout = np.asarray(Wout, np.float32)
    bout = np.asarray(bout, np.float32)

    nc = _get_program()
    in_maps = []
    for c in range(NCORES):
        b, g = divmod(c, GROUPS)
        # reference layout: Wqkv column j -> head j//192, role (j%192)//64
        idx_q = np.concatenate(
            [np.arange(h * 3 * HD, h * 3 * HD + HD)
             for h in range(g * HPG, (g + 1) * HPG)]
        )
        cols = np.concatenate([idx_q, idx_q + HD, idx_q + 2 * HD])
        w_loc = Wqkv[:, cols]
        b_loc = bqkv[cols][:, None]
        cs = slice(g * DG, (g + 1) * DG)
        in_maps.append({
            "xt": np.ascontiguousarray(x[b].T).astype(ml_dtypes.bfloat16),
            "wqkv": np.ascontiguousarray(w_loc).astype(ml_dtypes.bfloat16),
            "bqkv": np.ascontiguousarray(b_loc),
            "wout": np.ascontiguousarray(Wout[cs, :]),
        })

    res = run_bass_kernel_spmd(
        nc,
        in_maps,
        core_ids=list(range(NCORES)),
        trace=bool(int(os.environ.get("KERNEL_TRACE", "0"))),
    )
    LAST_RESULTS = res

    out = np.empty((B, S, E), np.float32)
    for b in range(B):
        acc = res.results[b * GROUPS]["y"].astype(np.float32)
        for g in range(1, GROUPS):
            acc += res.results[b * GROUPS + g]["y"].astype(np.float32)
        out[b] = acc + bout[None, :]
    return out
